# revision 1
# baseline (speedup 1.0000x reference)
"""CosineDistanceLoss kernel for Trainium2 (8 NeuronCores, Bass).

reference: mean_n(1 - sum_d feats[d,n] * warped_feats[d,n])
         = 1 - (1/N) * sum_{d,n} feats[d,n] * warped_feats[d,n]

The loss is a single global sum of the elementwise product, so any disjoint
partition of elements across cores is valid. We shard along D (rows): core c
gets rows [64c, 64c+64) of both tensors - contiguous views, zero host copy.
Each 64 x 65536 shard is a flat 4.19M-element buffer viewed as [128, 32768]
so SBUF tiles use all 128 partitions. The kernel streams [128, FCHUNK] chunks
of both tensors and runs one fused DVE scalar_tensor_tensor (elementwise mult
+ free-axis add-reduce via accum_out; the product tile is discarded through a
stride-0 broadcast output) per chunk, accumulating per-partition partial sums
into acc[:, j]. Host combines the 8 x 128 x NCHUNK partials in float64.

Two builders:
  - "raw":  hand-rolled semaphores (9 sems). Avoids TileContext's fixed
            overhead (~7us preamble scaffolding + ~10us epilogue semaphore
            storm), which matters at this ~100us scale.
  - "tile": TileContext version, kept for comparison / fallback.

This is DMA-bound: 32 MB/core over HBM at ~358 GB/s/core = ~90us floor.
"""

import numpy as np

import concourse.bacc as bacc
import concourse.mybir as mybir
from concourse.tile import TileContext
from concourse.bass_utils import run_bass_kernel_spmd

import os

D, N = 512, 65536
NCORES = 8
DSHARD = D // NCORES            # 64 rows per core
P = 128                         # SBUF partitions
M = DSHARD * N // P             # 32768 free elements per partition
FCHUNK = int(os.environ.get("COSLOSS_FCHUNK", "2048"))
NCHUNK = M // FCHUNK
SLOTS = int(os.environ.get("COSLOSS_SLOTS", "8"))
# Split the final chunk in two: after the last DMA lands, only half a chunk
# of DVE work remains exposed on the critical path.
TAIL_SPLIT = os.environ.get("COSLOSS_TAIL_SPLIT", "1") == "1"
# DRAM interpretation of each core's flat 4.19M-element shard:
#   strided: [128, M]; chunk j reads cols [j*F,(j+1)*F) - 128 streams strided
#            M*4 bytes apart, F*4 contiguous per stream.
#   seq:     [NCHUNK*128, F]; chunk j reads rows [j*128,(j+1)*128) - one fully
#            contiguous 128*F*4-byte block per DMA.
LAYOUT = os.environ.get("COSLOSS_LAYOUT", "strided")
# Issue feats-DMAs from the SP (sync) HWDGE ring and warped-DMAs from the
# Activation (scalar) ring: two descriptor generators in parallel.
TWO_RINGS = os.environ.get("COSLOSS_TWO_RINGS", "0") == "1"
DMA_BUFS = 3                    # tile-version double/triple buffering

IMPL = os.environ.get("COSLOSS_IMPL", "raw")

_CACHE = {}


def _in_shape():
    return [NCHUNK * P, FCHUNK] if LAYOUT == "seq" else [P, M]


def _declare_io(nc, ncols):
    shape = _in_shape()
    f_in = nc.declare_dram_parameter("feats", shape, mybir.dt.float32, isOutput=False)
    w_in = nc.declare_dram_parameter("warped", shape, mybir.dt.float32, isOutput=False)
    out = nc.declare_dram_parameter(
        "partial", [P, ncols], mybir.dt.float32, isOutput=True
    )
    return f_in, w_in, out


def _chunks():
    """List of (src_slicer, size). src_slicer(dram_ap) -> [128, size] source AP."""
    F = FCHUNK

    def strided(off, sz):
        return lambda t: t[:, off : off + sz]

    def seq(j, c0, sz):
        return lambda t: t[j * P : (j + 1) * P, c0 : c0 + sz]

    out = []
    for j in range(NCHUNK):
        last = j == NCHUNK - 1
        if last and TAIL_SPLIT and F >= 2048:
            # Quarter the last chunk: only ~F/4 columns of DVE work stay
            # exposed after the final DMA lands.
            q = F // 4
            for k in range(4):
                if LAYOUT == "seq":
                    out.append((seq(j, k * q, q), q))
                else:
                    out.append((strided(j * F + k * q, q), q))
        else:
            out.append((seq(j, 0, F) if LAYOUT == "seq" else strided(j * F, F), F))
    return out


def _build_raw():
    nc = bacc.Bacc(None)
    chunks = _chunks()
    nchunks = len(chunks)
    head = max(nchunks - 4, 0)  # acc cols written out early vs at the end
    f_in, w_in, out = _declare_io(nc, nchunks)
    F = FCHUNK
    assert SLOTS <= NCHUNK and 2 * SLOTS * F * 4 <= 176 * 1024, (SLOTS, F)

    with (
        nc.sbuf_tensor([P, SLOTS * F], mybir.dt.float32) as ftile,
        nc.sbuf_tensor([P, SLOTS * F], mybir.dt.float32) as wtile,
        nc.sbuf_tensor([P, nchunks], mybir.dt.float32) as acc,
        nc.sbuf_tensor([P, 1], mybir.dt.float32) as dummy,
    ):
        import contextlib

        with contextlib.ExitStack() as ctx:
            dsems = [
                ctx.enter_context(nc.semaphore(f"dsem{j}")) for j in range(nchunks)
            ]
            vsem = ctx.enter_context(nc.semaphore("vsem"))
            osem = ctx.enter_context(nc.semaphore("osem"))
            sem_nums = sorted(s.num for s in [*dsems, vsem, osem])
            assert sem_nums == list(
                range(sem_nums[0], sem_nums[0] + len(sem_nums))
            ), sem_nums
            sem_range = range(sem_nums[0], sem_nums[-1] + 1)

            # no_gpsimd_drain: this kernel issues no SWDGE work from gpsimd
            # (all DMAs ride the sync HWDGE ring, and the explicit dma_reset
            # covers our sem range), so the block exit can use the cheaper
            # drain + sem-only barrier.
            with nc.Block(no_gpsimd_drain=True) as block:

                @block.sync
                def _(sync):
                    for j, (src, sz) in enumerate(chunks):
                        s = j % SLOTS
                        if j >= SLOTS:
                            # WAR: slot s is being read by STT_{j-SLOTS};
                            # HWDGE issue is FIFO per ring, so this wait
                            # also orders this ring's later DMAs behind it.
                            sync.wait_ge(vsem, j - SLOTS + 1)
                        sync.dma_start(
                            ftile[:, s * F : s * F + sz], src(f_in)
                        ).then_inc(dsems[j], 16)
                        if not TWO_RINGS:
                            sync.dma_start(
                                wtile[:, s * F : s * F + sz], src(w_in)
                            ).then_inc(dsems[j], 16)
                    # Write out the bulk of acc early (overlaps the tail of
                    # the input stream); only the last columns stay on the
                    # post-stream critical path.
                    if head:
                        sync.wait_ge(vsem, head)
                        sync.dma_start(out[:, :head], acc[:, :head]).then_inc(
                            osem, 16
                        )
                    sync.wait_ge(vsem, nchunks)
                    sync.dma_start(out[:, head:], acc[:, head:]).then_inc(osem, 16)

                if TWO_RINGS:

                    @block.scalar
                    def _(scalar):
                        for j, (src, sz) in enumerate(chunks):
                            s = j % SLOTS
                            if j >= SLOTS:
                                scalar.wait_ge(vsem, j - SLOTS + 1)
                            scalar.dma_start(
                                wtile[:, s * F : s * F + sz], src(w_in)
                            ).then_inc(dsems[j], 16)

                @block.vector
                def _(vector):
                    for j, (src, sz) in enumerate(chunks):
                        s = j % SLOTS
                        vector.wait_ge(dsems[j], 32)
                        # out = (ft * 1.0) * wt (discarded via stride-0
                        # broadcast), accum_out = per-partition sum.
                        nc.vector.scalar_tensor_tensor(
                            dummy[:, :].broadcast_to((P, sz)),
                            ftile[:, s * F : s * F + sz],
                            1.0,
                            wtile[:, s * F : s * F + sz],
                            op0=mybir.AluOpType.mult,
                            op1=mybir.AluOpType.mult,
                            accum_out=acc[:, j : j + 1],
                        ).then_inc(vsem, 1)

                @block.gpsimd
                def _(gpsimd):
                    # osem at its final value implies both out-DMAs landed,
                    # which implies every earlier sem reached its final
                    # value. Reset them so the NEFF is safe to re-execute.
                    gpsimd.wait_ge(osem, 32 if head else 16)
                    gpsimd.dma_reset(sem_range)
                    gpsimd.sem_clear(sem_range)

    nc.finalize()
    return nc


def _build_tile():
    nc = bacc.Bacc(None)
    f_in, w_in, out = _declare_io(nc, NCHUNK)

    with TileContext(nc) as tc:
        with (
            tc.tile_pool(name="accp", bufs=1) as accp,
            tc.tile_pool(name="fp", bufs=DMA_BUFS) as fp,
            tc.tile_pool(name="wp", bufs=DMA_BUFS) as wp,
            tc.tile_pool(name="dp", bufs=NCHUNK) as dp,
        ):
            acc = accp.tile([P, NCHUNK], mybir.dt.float32)
            for j in range(NCHUNK):
                ft = fp.tile([P, FCHUNK], mybir.dt.float32)
                wt = wp.tile([P, FCHUNK], mybir.dt.float32)
                nc.sync.dma_start(ft[:, :], f_in[:, j * FCHUNK : (j + 1) * FCHUNK])
                nc.sync.dma_start(wt[:, :], w_in[:, j * FCHUNK : (j + 1) * FCHUNK])
                dummy = dp.tile([P, 1], mybir.dt.float32)
                nc.vector.scalar_tensor_tensor(
                    dummy.broadcast_to((P, FCHUNK)),
                    ft[:, :],
                    1.0,
                    wt[:, :],
                    op0=mybir.AluOpType.mult,
                    op1=mybir.AluOpType.mult,
                    accum_out=acc[:, j : j + 1],
                )
            nc.sync.dma_start(out[:, :], acc[:, :])

    nc.finalize()
    return nc


def _get_nc(impl=None):
    impl = impl or IMPL
    if impl not in _CACHE:
        _CACHE[impl] = _build_raw() if impl == "raw" else _build_tile()
    return _CACHE[impl]


def _run(feats, warped_feats, impl=None, **spmd_kwargs):
    feats = np.ascontiguousarray(np.asarray(feats), dtype=np.float32)
    warped = np.ascontiguousarray(np.asarray(warped_feats), dtype=np.float32)
    assert feats.shape == (D, N) and warped.shape == (D, N)

    shape = tuple(_in_shape())
    in_maps = [
        {
            "feats": feats[c * DSHARD : (c + 1) * DSHARD].reshape(shape),
            "warped": warped[c * DSHARD : (c + 1) * DSHARD].reshape(shape),
        }
        for c in range(NCORES)
    ]
    return run_bass_kernel_spmd(
        _get_nc(impl), in_maps, core_ids=list(range(NCORES)), **spmd_kwargs
    )


def kernel(feats, warped_feats):
    res = _run(feats, warped_feats)
    total = 0.0
    for r in res.results:
        total += float(r["partial"].astype(np.float64).sum())
    return np.array(1.0 - total / N, dtype=np.float32)

